# revision 39
# baseline (speedup 1.0000x reference)
"""Decision Transformer on 8 Trainium2 NeuronCores.

Sharding: batch(4) x 2-way tensor parallel (attention-head split; FFN and Wo
computed redundantly on both pair cores to avoid AllReduce). Core c: batch
c//2, head-shard c%2. All cores run one SPMD instruction stream; shard
differences live in the input data. One fp8 pair-AllGather per (layer, chunk).

On-chip layout: residual stream x is transposed ([D, S]) in bf16; a quantized
fp8 copy xh feeds all weight matmuls, which run in fp8 DoubleRow mode
(2 k-tiles per matmul, 0.5 PE cycles/row). Weights are host-prescaled into
fp8 range and rescaled on PSUM evacuation. LayerNorm stats use ones-vector
matmuls; normalization applies rank-1 broadcast tiles (rstd, mean*rstd) plus
a per-partition gain/bias pass. Attention keeps logits [key, query], windows
the exp/causal-mask work to the live region, and defers softmax normalization
via a ones-column in V. Residual adds are fused into PSUM evacuation with
scalar_tensor_tensor. Emission is software-pipelined across layers so
collectives hide under the next chunk's compute.
"""

import numpy as np

import concourse.bass as bass
import concourse.mybir as mybir
import concourse.tile as tile
from concourse import bacc
from concourse.bass_utils import run_bass_kernel_spmd
from concourse.masks import make_identity

F32 = mybir.dt.float32
BF16 = mybir.dt.bfloat16
FP8 = mybir.dt.float8e4
I32 = mybir.dt.int32
AF = mybir.ActivationFunctionType
OP = mybir.AluOpType
DR = mybir.MatmulPerfMode.DoubleRow

N, L, D = 4, 512, 768
STATE, ACT_DIM = 17, 6
H, KD = 12, 64
FF = 2048
NL = 4
MAXT = 4096

S = 3 * L            # 1536 tokens
DT = D // 128        # 6 d-tiles
DP = DT // 2         # 3 d-tile pairs
CW = 512             # chunk width (tokens)
NCH = S // CW        # 3 chunks
KT = S // 128        # 12 k-tiles
HD = H // 2          # 6 heads per core
HP = HD // 2         # 3 head pairs
FFC = FF // 128      # 16 ff col-tiles
EPS = 1e-5

SW = 64.0            # prescale for wq/wk/wv/wo/w2
SW1 = 32.0           # prescale for w1 (relu path)

REPLICA_GROUPS = [[0, 1], [2, 3], [4, 5], [6, 7]]


def build_nc(trivial_gb=True):
    nc = bacc.Bacc("TRN2", target_bir_lowering=False, debug=False, num_devices=8)

    d_rT = nc.dram_tensor("rT", [2, L], BF16, kind="ExternalInput")
    d_sT = nc.dram_tensor("sT", [STATE + 1, L], BF16, kind="ExternalInput")
    d_aT = nc.dram_tensor("aT", [ACT_DIM + 1, L], BF16, kind="ExternalInput")
    d_tix = nc.dram_tensor("tix", [L, 1], I32, kind="ExternalInput")
    d_emb = nc.dram_tensor("emb", [MAXT, D], F32, kind="ExternalInput")
    d_wr = nc.dram_tensor("wr", [2, D], BF16, kind="ExternalInput")
    d_ws = nc.dram_tensor("ws", [STATE + 1, D], BF16, kind="ExternalInput")
    d_wa = nc.dram_tensor("wa", [ACT_DIM + 1, D], BF16, kind="ExternalInput")
    d_lng = nc.dram_tensor("lng", [9, D], F32, kind="ExternalInput")
    d_lnb = nc.dram_tensor("lnb", [9, D], F32, kind="ExternalInput")
    d_wq = nc.dram_tensor("wq", [NL, D, HD * KD], BF16, kind="ExternalInput")
    d_wk = nc.dram_tensor("wk", [NL, D, HD * KD], BF16, kind="ExternalInput")
    d_wv = nc.dram_tensor("wv", [NL, D, HD * KD], BF16, kind="ExternalInput")
    d_wo = nc.dram_tensor("wo", [NL, H * KD, D], FP8, kind="ExternalInput")
    d_w1 = nc.dram_tensor("w1", [NL, D, FF], BF16, kind="ExternalInput")
    d_b1s = nc.dram_tensor("b1s", [NL, FF], F32, kind="ExternalInput")
    d_w2 = nc.dram_tensor("w2", [NL, FF, D], BF16, kind="ExternalInput")
    d_b2r = nc.dram_tensor("b2r", [1, NL * D], BF16, kind="ExternalInput")
    d_wpa = nc.dram_tensor("wpa", [D, ACT_DIM], BF16, kind="ExternalInput")
    d_bpa = nc.dram_tensor("bpa", [1, ACT_DIM], F32, kind="ExternalInput")
    d_out = nc.dram_tensor("outT", [ACT_DIM, L], F32, kind="ExternalOutput")

    from contextlib import ExitStack
    with tile.TileContext(nc) as tc:
        with ExitStack() as _es:
            pp = _es.enter_context(tc.tile_pool(name="persist", bufs=1))
            wqkvp = _es.enter_context(tc.tile_pool(name="wqkv", bufs=1))
            wop = _es.enter_context(tc.tile_pool(name="wop", bufs=2))
            wffp = _es.enter_context(tc.tile_pool(name="wff", bufs=1))
            qtp = _es.enter_context(tc.tile_pool(name="qt", bufs=2))
            attp = _es.enter_context(tc.tile_pool(name="att", bufs=2))
            gatp = _es.enter_context(tc.tile_pool(name="gatp", bufs=2))
            posp = _es.enter_context(tc.tile_pool(name="posp", bufs=2))
            prp = _es.enter_context(tc.tile_pool(name="pr", bufs=4))
            htp = _es.enter_context(tc.tile_pool(name="htp", bufs=1))
            scr = _es.enter_context(tc.tile_pool(name="scr", bufs=3))
            abp = _es.enter_context(tc.tile_pool(name="abp", bufs=2))
            rowsp = _es.enter_context(tc.tile_pool(name="rows", bufs=3))
            smallp = _es.enter_context(tc.tile_pool(name="small", bufs=4))
            psA = _es.enter_context(tc.tile_pool(name="psA", bufs=2, space="PSUM"))
            psLG = _es.enter_context(tc.tile_pool(name="pslg", bufs=2, space="PSUM"))
            psST = _es.enter_context(tc.tile_pool(name="psst", bufs=1, space="PSUM"))
            psAB = _es.enter_context(tc.tile_pool(name="psab", bufs=1, space="PSUM"))
            psPV = _es.enter_context(tc.tile_pool(name="pspv", bufs=1, space="PSUM"))
            drp = _es.enter_context(tc.tile_pool(name="dram", bufs=3, space="DRAM"))
            # ---- persistent tiles ----
            x = pp.tile([128, DT, S], BF16)          # residual stream, transposed
            kT = pp.tile([128, HP, S], BF16)         # K^T (own heads)
            v = pp.tile([128, KT, HD * 128], FP8)    # V rows + ones col + pad per head
            xs = pp.tile([128, DT, L], BF16)         # last layer: packed state tokens
            ident = pp.tile([128, 128], F32)
            ones_col = pp.tile([128, 1], BF16)       # stat matmul lhsT
            onesP = pp.tile([1, 128], BF16)          # broadcast lhsT
            ones_row = pp.tile([1, CW], BF16)        # bias-row matmul rhs
            lng_sb = pp.tile([128, 9, DT], F32)
            lnb_sb = pp.tile([128, 9, DT], F32)
            b1s_sb = pp.tile([128, NL, FFC], F32)
            bpa_sb = pp.tile([ACT_DIM, 1], F32)
            wpa_sb = pp.tile([128, DT, ACT_DIM], BF16)
            wr_sb = pp.tile([2, D], BF16)
            ws_sb = pp.tile([STATE + 1, D], BF16)
            wa_sb = pp.tile([ACT_DIM + 1, D], BF16)
            rT_sb = pp.tile([2, L], BF16)
            sT_sb = pp.tile([STATE + 1, L], BF16)
            aT_sb = pp.tile([ACT_DIM + 1, L], BF16)

            magic_row = pp.tile([1, CW], I32)
            nc.vector.memset(magic_row, 0x5F3759DF)
            expb_sb = pp.tile([128, 1], F32)
            nc.vector.memset(expb_sb, -2.0)
            make_identity(nc, ident)
            nc.vector.memset(ones_col, 1.0)
            nc.vector.memset(onesP, 1.0)
            nc.vector.memset(ones_row, 1.0)

            pos_tiles = []
            for r in range(L // 128):
                tix_sb = smallp.tile([128, 1], I32, tag="tix", name=f"tix{r}")
                nc.sync.dma_start(out=tix_sb, in_=d_tix.ap()[r * 128 : (r + 1) * 128, :])
                pos = posp.tile([128, D], F32, tag="pos", bufs=4, name=f"pos{r}")
                nc.gpsimd.indirect_dma_start(
                    out=pos, out_offset=None, in_=d_emb.ap(),
                    in_offset=bass.IndirectOffsetOnAxis(ap=tix_sb[:, :1], axis=0))
                pos_tiles.append(pos)
            magic_row = pp.tile([1, CW], I32)
            nc.vector.memset(magic_row, 0x5F3759DF)
            make_identity(nc, ident)
            nc.vector.memset(ones_col, 1.0)
            nc.vector.memset(onesP, 1.0)
            nc.vector.memset(ones_row, 1.0)
            nc.sync.dma_start(out=wr_sb, in_=d_wr.ap())
            nc.sync.dma_start(out=ws_sb, in_=d_ws.ap())
            nc.sync.dma_start(out=wa_sb, in_=d_wa.ap())
            nc.sync.dma_start(out=rT_sb, in_=d_rT.ap())
            nc.sync.dma_start(out=sT_sb, in_=d_sT.ap())
            nc.sync.dma_start(out=aT_sb, in_=d_aT.ap())
            nc.sync.dma_start(out=lng_sb, in_=d_lng.ap().rearrange("g (t p) -> p g t", p=128))
            nc.sync.dma_start(out=lnb_sb, in_=d_lnb.ap().rearrange("g (t p) -> p g t", p=128))
            nc.sync.dma_start(out=b1s_sb, in_=d_b1s.ap().rearrange("l (t p) -> p l t", p=128))
            nc.sync.dma_start(out=bpa_sb, in_=d_bpa.ap().rearrange("o c -> c o"))
            nc.sync.dma_start(out=wpa_sb, in_=d_wpa.ap().rearrange("(t p) c -> p t c", p=128))

            nc.gpsimd.memset(v, 0.0)
            nc.gpsimd.memset(
                v.rearrange("p k (h w) -> p k h w", w=128)[:, :, :, 64:65], 1.0)

            # ---- per-layer weight tiles ----
            def load_qkv(li):
                wq_sb = wqkvp.tile([128, DT, HD * KD], BF16, tag="wq", name=f"wq{li}")
                wk_sb = wqkvp.tile([128, DT, HD * KD], BF16, tag="wk", name=f"wk{li}")
                wv_sb = wqkvp.tile([128, DT, HD * KD], BF16, tag="wv", name=f"wv{li}")
                nc.sync.dma_start(out=wq_sb, in_=d_wq.ap()[li].rearrange("(t p) c -> p t c", p=128))
                nc.sync.dma_start(out=wk_sb, in_=d_wk.ap()[li].rearrange("(t p) c -> p t c", p=128))
                nc.sync.dma_start(out=wv_sb, in_=d_wv.ap()[li].rearrange("(t p) c -> p t c", p=128))
                return wq_sb, wk_sb, wv_sb

            def load_wo(li):
                wo_sb = wop.tile([128, 2 * HP, D], FP8, tag="wo", name=f"wo{li}")
                nc.sync.dma_start(out=wo_sb, in_=d_wo.ap()[li].rearrange("(t p) c -> p t c", p=128))
                return wo_sb

            def load_ffn(li):
                w1_sb = wffp.tile([128, DT, FF], BF16, tag="w1", name=f"w1{li}")
                w2_sb = wffp.tile([128, FFC, D], BF16, tag="w2", name=f"w2{li}")
                b2l_sb = wffp.tile([1, D], BF16, tag="b2l", bufs=2, name=f"b2l{li}")
                nc.sync.dma_start(out=w1_sb, in_=d_w1.ap()[li].rearrange("(t p) c -> p t c", p=128))
                nc.sync.dma_start(out=w2_sb, in_=d_w2.ap()[li].rearrange("(t p) c -> p t c", p=128))
                nc.sync.dma_start(out=b2l_sb, in_=d_b2r.ap()[:, li * D : (li + 1) * D])
                return w1_sb, w2_sb, b2l_sb

            # ---- helpers ----
            def xcols(c):
                cs = slice(c * CW, (c + 1) * CW)
                return lambda dt: x[:, dt, cs]

            def x_state(dt):
                return x[:, dt, :].rearrange("p (j k) -> p k j", k=3)[:, 1, :]

            def layer_norm_gen(gi, xc, W):
                """xc(dt)->[128,W] bf16, normalized in place."""
                st = psST.tile([33, W], F32, tag="st", name="st")
                for dt in range(DT):
                    sq = scr.tile([128, W], BF16, tag="scr", name="sq")
                    nc.scalar.activation(out=sq, in_=xc(dt), func=AF.Square, bias=0.0, scale=1.0)
                    nc.tensor.matmul(st[0:1, :], lhsT=ones_col, rhs=xc(dt),
                                     start=(dt == 0), stop=(dt == DT - 1))
                    nc.tensor.matmul(st[32:33, :], lhsT=ones_col, rhs=sq,
                                     start=(dt == 0), stop=(dt == DT - 1))
                mrow = rowsp.tile([1, W], F32, tag="rowf", name="mrow")
                nc.vector.tensor_scalar(out=mrow, in0=st[0:1, :], scalar1=1.0 / D,
                                        scalar2=None, op0=OP.mult)
                m2 = rowsp.tile([1, W], F32, tag="rowf", name="m2")
                nc.vector.tensor_tensor(out=m2, in0=mrow, in1=mrow, op=OP.mult)
                ve = rowsp.tile([1, W], F32, tag="rowf", name="ve")
                nc.vector.scalar_tensor_tensor(out=ve, in0=st[32:33, :], scalar=1.0 / D,
                                               in1=m2, op0=OP.mult, op1=OP.subtract)
                # rsqrt via bit-trick seed + 2 Newton iterations (DVE only,
                # avoids Act sqrt/ln which would thrash the activation tables)
                yi = rowsp.tile([1, W], I32, tag="rowf", name="yi")
                nc.vector.tensor_scalar(out=yi, in0=ve.bitcast(I32), scalar1=1,
                                        scalar2=None, op0=OP.logical_shift_right)
                nc.vector.tensor_tensor(out=yi, in0=magic_row[0:1, 0:W], in1=yi,
                                        op=OP.subtract)
                y = yi.bitcast(F32)
                t = rowsp.tile([1, W], F32, tag="rowf", name="t")
                nc.vector.tensor_tensor(out=t, in0=y, in1=y, op=OP.mult)
                nc.vector.tensor_tensor(out=t, in0=t, in1=ve, op=OP.mult)
                nc.vector.tensor_scalar(out=t, in0=t, scalar1=-0.5, scalar2=1.5,
                                        op0=OP.mult, op1=OP.add)
                rstd = rowsp.tile([1, W], BF16, tag="rowb", name="rstd")
                with nc.allow_low_precision(reason="bf16 rstd feeds bf16 matmul"):
                    nc.vector.tensor_tensor(out=rstd, in0=y, in1=t, op=OP.mult)
                beta = rowsp.tile([1, W], BF16, tag="rowb", name="beta")
                nc.vector.scalar_tensor_tensor(out=beta, in0=st[0:1, :], scalar=1.0 / D,
                                               in1=rstd, op0=OP.mult, op1=OP.mult)
                yield
                ab = psAB.tile([128, 2, CW], F32, tag="ab", name="ab")
                nc.tensor.matmul(ab[:, 0, 0:W], lhsT=onesP, rhs=rstd, start=True, stop=True)
                nc.tensor.matmul(ab[:, 1, 0:W], lhsT=onesP, rhs=beta, start=True, stop=True)
                a_sb = abp.tile([128, W], BF16, tag="a_sb", name="a_sb")
                b_sb = abp.tile([128, W], BF16, tag="b_sb", name="b_sb")
                nc.scalar.activation(out=a_sb, in_=ab[:, 0, 0:W], func=AF.Identity,
                                     bias=0.0, scale=1.0)
                nc.scalar.activation(out=b_sb, in_=ab[:, 1, 0:W], func=AF.Identity,
                                     bias=0.0, scale=1.0)
                yield
                for dt in range(DT):
                    if trivial_gb:
                        t1 = scr.tile([128, W], BF16, tag="scr", name="t1")
                        nc.vector.tensor_tensor(out=t1, in0=xc(dt), in1=a_sb, op=OP.mult)
                        nc.vector.tensor_tensor(out=xc(dt), in0=t1, in1=b_sb, op=OP.subtract)
                    else:
                        t1 = scr.tile([128, W], BF16, tag="scr", name="t1")
                        nc.vector.tensor_tensor(out=t1, in0=xc(dt), in1=a_sb, op=OP.mult)
                        nc.vector.tensor_tensor(out=t1, in0=t1, in1=b_sb, op=OP.subtract)
                        nc.vector.tensor_scalar(out=xc(dt), in0=t1,
                                                scalar1=lng_sb[:, gi, dt : dt + 1],
                                                scalar2=lnb_sb[:, gi, dt : dt + 1],
                                                op0=OP.mult, op1=OP.add)
                    if dt % 3 == 2:
                        yield

            def kv_gen(wk_sb, wv_sb, c):
                cs = slice(c * CW, (c + 1) * CW)
                for hp in range(HP):
                    pk = psA.tile([128, CW], F32, tag="mm", name="pk")
                    for dt in range(DT):
                        nc.tensor.matmul(pk, lhsT=wk_sb[:, dt, hp * 128 : (hp + 1) * 128],
                                         rhs=x[:, dt, cs],
                                         start=(dt == 0), stop=(dt == DT - 1))
                    nc.scalar.activation(out=kT[:, hp, cs], in_=pk, func=AF.Identity,
                                         bias=0.0, scale=1.0)
                    yield
                for j in range(4):
                    kt = 4 * c + j
                    pv_ = psA.tile([128, HD * KD], F32, tag="mm", name="pv_")
                    for dt in range(DT):
                        nc.tensor.matmul(pv_, lhsT=x[:, dt, kt * 128 : (kt + 1) * 128],
                                         rhs=wv_sb[:, dt, :],
                                         start=(dt == 0), stop=(dt == DT - 1))
                    nc.scalar.activation(
                        out=v[:, kt, :].rearrange("p (h w) -> p h w", w=128)[:, :, 0:64],
                        in_=pv_.rearrange("p (h w) -> p h w", w=64),
                        func=AF.Identity, bias=0.0, scale=1.0)
                    if j % 2 == 1:
                        yield

            def attn_gen(wq_sb, qsrc, npair, masks, W, out_attnT):
                """masks: list per kt of None or (dead, w0, base, step) for the
                affine window [w0, w0+winw) with iota = base + step*i - p."""
                qTc = qtp.tile([128, HP, W], BF16, tag="qT", name="qTc")
                for hp in range(HP):
                    pq = psA.tile([128, W], F32, tag="mm", name="pq")
                    for dt in range(DT):
                        nc.tensor.matmul(pq, lhsT=wq_sb[:, dt, hp * 128 : (hp + 1) * 128],
                                         rhs=qsrc(dt), start=(dt == 0), stop=(dt == DT - 1))
                    nc.vector.tensor_copy(out=qTc[:, hp, :], in_=pq)
                yield
                for hd in range(HD):
                    hp, hi = hd // 2, hd % 2
                    prow = slice(64 * hi, 64 * hi + 64)
                    pv = psPV.tile([128, W], F32, tag="pv", name="pv")
                    for ip in range(npair):
                        pr2 = prp.tile([128, 2, W], FP8, tag="pr", name="pr2")
                        for j in (0, 1):
                            kt = 2 * ip + j
                            lgp = psLG.tile([128, CW], F32, tag="lg", name="lgp")
                            nc.tensor.matmul(lgp[:, 0:W],
                                             lhsT=kT[prow, hp, kt * 128 : (kt + 1) * 128],
                                             rhs=qTc[prow, hp, :], start=True, stop=True)
                            m_ = masks[kt]
                            dead, w0, bse, stp, winw = (0, 0, 0, 1, 0) if m_ is None else m_
                            if dead > 0:
                                nc.gpsimd.memset(pr2[:, j, 0:dead], 0.0)
                            if dead < W:
                                nc.scalar.activation(out=pr2[:, j, dead:W],
                                                     in_=lgp[:, dead:W], func=AF.Exp,
                                                     bias=expb_sb, scale=float(KD) ** -0.5)
                            if winw > 0:
                                nc.gpsimd.affine_select(
                                    out=pr2[:, j, w0 : w0 + winw],
                                    in_=pr2[:, j, w0 : w0 + winw],
                                    compare_op=OP.is_ge, fill=0.0,
                                    base=bse, channel_multiplier=-1,
                                    pattern=[[stp, winw]])
                        nc.tensor.matmul(pv, lhsT=v[:, 2 * ip : 2 * ip + 2, hd * 128 : (hd + 1) * 128],
                                         rhs=pr2, start=(ip == 0), stop=(ip == npair - 1),
                                         perf_mode=DR)
                    nc.scalar.activation(out=out_attnT[prow, hp, :], in_=pv[0:64, :],
                                         func=AF.Identity, bias=0.0, scale=1.0)
                    rc = rowsp.tile([1, W], BF16, tag="rowb", name="rc")
                    with nc.allow_low_precision(reason="bf16 softmax denom recip"):
                        nc.vector.reciprocal(out=rc, in_=pv[64:65, :])
                    bc = psA.tile([64, W], F32, tag="mm", name="bc")
                    nc.tensor.matmul(bc, lhsT=onesP[:, 0:64], rhs=rc, start=True, stop=True)
                    nc.vector.tensor_tensor(out=out_attnT[prow, hp, :],
                                            in0=out_attnT[prow, hp, :],
                                            in1=bc, op=OP.mult)
                    yield

            def stage_allgather(attnT, W):
                ag_in = drp.tile([HP * 128, W], FP8, tag="agin", name="ag_in")
                ag_o = drp.tile([2 * HP * 128, W], FP8, tag="agout", name="ag_o")
                nc.sync.dma_start(out=ag_in.rearrange("(t p) c -> p t c", p=128), in_=attnT)
                nc.gpsimd.collective_compute(
                    "AllGather", OP.bypass, replica_groups=REPLICA_GROUPS,
                    ins=[ag_in.opt()], outs=[ag_o.opt()])
                return ag_o

            # full-layer causal masks per chunk: diagonal k-tiles get a
            # 128-wide affine window at w0=(kt-4c)*128, iota = i - p
            def chunk_masks(c):
                ms = []
                for kt in range(4 * (c + 1)):
                    if kt >= 4 * c:
                        w0 = (kt - 4 * c) * 128
                        ms.append((w0, w0, 0, 1, min(128, CW - w0)))
                    else:
                        ms.append(None)
                return ms

            def wo_residual_gen(wo_sb, ag_o, xc, W):
                gat = gatp.tile([128, 2 * HP, W], FP8, tag="gat", name="gat")
                nc.sync.dma_start(out=gat, in_=ag_o.rearrange("(t p) c -> p t c", p=128))
                for dc in range(DT):
                    py = psA.tile([128, W], F32, tag="mm", name="py")
                    for h2 in range(HP):
                        nc.tensor.matmul(py, lhsT=wo_sb[:, 2 * h2 : 2 * h2 + 2, dc * 128 : (dc + 1) * 128],
                                         rhs=gat[:, 2 * h2 : 2 * h2 + 2, :],
                                         start=(h2 == 0), stop=(h2 == HP - 1), perf_mode=DR)
                    nc.vector.scalar_tensor_tensor(out=xc(dc), in0=py, scalar=1.0 / SW,
                                                   in1=xc(dc), op0=OP.mult, op1=OP.add)
                    if dc % 2 == 1:
                        yield

            def ffn_gen(li, w1_sb, w2_sb, b2l_sb, xc, W):
                ht = htp.tile([128, FFC, W], BF16, tag="ht", name="ht")
                for f in range(FFC):
                    ph = psA.tile([128, W], F32, tag="mm", name="ph")
                    for dt in range(DT):
                        nc.tensor.matmul(ph, lhsT=w1_sb[:, dt, f * 128 : (f + 1) * 128],
                                         rhs=xc(dt), start=(dt == 0), stop=(dt == DT - 1))
                    nc.scalar.activation(out=ht[:, f, :], in_=ph, func=AF.Relu,
                                         bias=b1s_sb[:, li, f : f + 1], scale=1.0)
                    if f % 4 == 3:
                        yield
                for dc in range(DT):
                    ps2 = psA.tile([128, W], F32, tag="mm", name="ps2")
                    nc.tensor.matmul(ps2, lhsT=b2l_sb[0:1, dc * 128 : (dc + 1) * 128],
                                     rhs=ones_row[0:1, 0:W], start=True, stop=False)
                    for f in range(FFC):
                        nc.tensor.matmul(ps2, lhsT=w2_sb[:, f, dc * 128 : (dc + 1) * 128],
                                         rhs=ht[:, f, :],
                                         start=False, stop=(f == FFC - 1))
                    nc.vector.scalar_tensor_tensor(out=xc(dc), in0=ps2, scalar=1.0,
                                                   in1=xc(dc), op0=OP.mult, op1=OP.add)
                    if dc % 2 == 1:
                        yield

            def mlp_gen(li, wo_sb, wff3, ag_o, xc, W):
                yield from wo_residual_gen(wo_sb, ag_o, xc, W)
                yield from layer_norm_gen(1 + li, xc, W)
                yield from ffn_gen(li, wff3[0], wff3[1], wff3[2], xc, W)
                yield from layer_norm_gen(5 + li, xc, W)

            def drain(g):
                for _ in g:
                    pass

            def co_emit(*specs):
                # specs: (generator, start_round) pairs, round-robin emission
                gens = [[g, start, False] for g, start in specs]
                rnd = 0
                while not all(g[2] for g in gens):
                    for g in gens:
                        if g[2] or rnd < g[1]:
                            continue
                        try:
                            next(g[0])
                        except StopIteration:
                            g[2] = True
                    rnd += 1

            # ---- embedding ----
            wq0, wk0, wv0 = load_qkv(0)
            wo0 = load_wo(0)
            wff0 = load_ffn(0)

            def x_kind(dt, kind):
                return x[:, dt, :].rearrange("p (j k) -> p k j", k=3)[:, kind, :]

            for dt in range(DT):
                for w_sb, t_sb, kind in ((wr_sb, rT_sb, 0), (ws_sb, sT_sb, 1), (wa_sb, aT_sb, 2)):
                    pe = psA.tile([128, L], F32, tag="mm", name="pe")
                    for r in range(L // 128):
                        nc.tensor.matmul(pe[:, r * 128 : (r + 1) * 128],
                                         lhsT=pos_tiles[r][:, dt * 128 : (dt + 1) * 128],
                                         rhs=ident, start=(r == 0), stop=False,
                                         is_transpose=True)
                    nc.tensor.matmul(pe, lhsT=w_sb[:, dt * 128 : (dt + 1) * 128], rhs=t_sb,
                                     start=False, stop=True)
                    nc.scalar.activation(out=x_kind(dt, kind), in_=pe, func=AF.Identity,
                                         bias=0.0, scale=1.0)

            # ---- pipelined layers ----
            # Two concurrent emission streams per phase-pair (attention of one
            # chunk interleaved with the MLP of another) so every engine's
            # 4-deep wait-queue window always holds ready work.
            QKV = {0: (wq0, wk0, wv0)}
            WO = {0: wo0}
            W12 = {0: wff0}
            ag_pending = {}
            ag_s = {}
            HWL = CW // 2

            def chain2(*gens):
                for g in gens:
                    yield from g

            def attn_stage_gen(li, c):
                attnT = attp.tile([128, HP, CW], FP8, tag="attnT", name="attnT")
                yield from attn_gen(QKV[li][0], xcols(c), 2 * (c + 1), chunk_masks(c), CW, attnT)
                ag_pending[(li, c)] = stage_allgather(attnT, CW)

            def state_masks(hw0, W, nkt):
                ms = []
                for kt in range(nkt):
                    t0 = max(0, (kt * 128 + 1) // 3)     # first possibly-live col (global)
                    dead = min(W, max(0, t0 - hw0))
                    w0 = dead
                    winw = min(44, W - w0)
                    base = 3 * (hw0 + w0) - kt * 128 + 1
                    if dead >= W:
                        ms.append((W, 0, 0, 3, 0))
                    else:
                        ms.append((dead, w0, base, 3, winw))
                return ms

            def compact_gen(hw0, W):
                hs = slice(hw0, hw0 + W)
                for dt in range(DT):
                    nc.gpsimd.tensor_copy(out=xs[:, dt, hs], in_=x_state(dt)[:, hs])
                    if dt % 3 == 2:
                        yield

            def attn_state_gen(half):
                hw0 = half * HWL
                hs = slice(hw0, hw0 + HWL)
                nkt = 6 if half == 0 else 12
                li = NL - 1
                attnT = attp.tile([128, HP, HWL], FP8, tag="attnTs", name="attnTs")
                yield from attn_gen(QKV[li][0], (lambda dt: xs[:, dt, hs]),
                                    nkt // 2, state_masks(hw0, HWL, nkt), HWL, attnT)
                ag_s[half] = stage_allgather(attnT, HWL)

            def mlp_state_head_gen(half):
                li = NL - 1
                hw0 = half * HWL
                hs = slice(hw0, hw0 + HWL)
                xc = lambda dt: xs[:, dt, hs]
                yield from mlp_gen(li, WO[li], W12[li], ag_s[half], xc, HWL)
                po = psA.tile([ACT_DIM, HWL], F32, tag="mm", name="po")
                for dt in range(DT):
                    nc.tensor.matmul(po, lhsT=wpa_sb[:, dt, :], rhs=xs[:, dt, hs],
                                     start=(dt == 0), stop=(dt == DT - 1))
                ot = scr.tile([ACT_DIM, HWL], F32, tag="ot", bufs=2, name="ot")
                nc.scalar.activation(out=ot, in_=po, func=AF.Identity, bias=bpa_sb, scale=1.0)
                nc.sync.dma_start(out=d_out.ap()[:, hs], in_=ot)

            drain(layer_norm_gen(0, xcols(0), CW))
            drain(kv_gen(wk0, wv0, 0))
            co_emit((attn_stage_gen(0, 0), 0),
                    (chain2(layer_norm_gen(0, xcols(1), CW),
                            kv_gen(wk0, wv0, 1),
                            layer_norm_gen(0, xcols(2), CW)), 0))

            for li in range(NL - 1):
                co_emit((attn_stage_gen(li, 1), 0),
                        (mlp_gen(li, WO[li], W12[li], ag_pending[(li, 0)], xcols(0), CW), 3),
                        (kv_gen(QKV[li][1], QKV[li][2], 2), 0))
                QKV[li + 1] = load_qkv(li + 1)
                WO[li + 1] = load_wo(li + 1)
                co_emit((attn_stage_gen(li, 2), 0),
                        (mlp_gen(li, WO[li], W12[li], ag_pending[(li, 1)], xcols(1), CW), 3))
                W12[li + 1] = load_ffn(li + 1)
                if li + 1 < NL - 1:
                    co_emit((chain2(kv_gen(QKV[li + 1][1], QKV[li + 1][2], 0),
                                    attn_stage_gen(li + 1, 0)), 0),
                            (mlp_gen(li, WO[li], W12[li], ag_pending[(li, 2)], xcols(2), CW), 3),
                            (kv_gen(QKV[li + 1][1], QKV[li + 1][2], 1), 0))
                else:
                    lz = li + 1
                    co_emit((chain2(kv_gen(QKV[lz][1], QKV[lz][2], 0),
                                    kv_gen(QKV[lz][1], QKV[lz][2], 1),
                                    compact_gen(0, HWL),
                                    attn_state_gen(0)), 0),
                            (mlp_gen(li, WO[li], W12[li], ag_pending[(li, 2)], xcols(2), CW), 3))
                    co_emit((chain2(kv_gen(QKV[lz][1], QKV[lz][2], 2),
                                    compact_gen(HWL, HWL),
                                    attn_state_gen(1)), 0),
                            (mlp_state_head_gen(0), 3))
                    drain(mlp_state_head_gen(1))

    nc.compile()
    return nc


_NC_CACHE = {}


def _get_nc(trivial_gb=True):
    if trivial_gb not in _NC_CACHE:
        _NC_CACHE[trivial_gb] = build_nc(trivial_gb)
    return _NC_CACHE[trivial_gb]


def _make_in_maps(inputs):
    import ml_dtypes
    FP8NP = ml_dtypes.float8_e4m3
    BF16NP = ml_dtypes.bfloat16

    f32 = lambda a: np.ascontiguousarray(np.asarray(a, dtype=np.float32))
    bf = lambda a: np.ascontiguousarray(np.asarray(a, dtype=np.float32).astype(BF16NP))
    fp8 = lambda a, s: np.ascontiguousarray(
        (np.asarray(a, dtype=np.float32) * s).astype(FP8NP))

    R, s, a, t = f32(inputs["R"]), f32(inputs["s"]), f32(inputs["a"]), np.asarray(inputs["t"])
    ones = np.ones((1, L), np.float32)
    lng = np.concatenate([f32(inputs["ln0_g"])[None], f32(inputs["ln1_g"]), f32(inputs["ln2_g"])], 0)
    lnb = np.concatenate([f32(inputs["ln0_b"])[None], f32(inputs["ln1_b"]), f32(inputs["ln2_b"])], 0)
    wr = np.concatenate([f32(inputs["Wr"]), f32(inputs["br"])[None]], 0)
    ws = np.concatenate([f32(inputs["Ws"]), f32(inputs["bs"])[None]], 0)
    wa = np.concatenate([f32(inputs["Wa"]), f32(inputs["ba"])[None]], 0)
    emb = f32(inputs["embed_t"])
    Wq, Wk, Wv, Wo = inputs["Wq"], inputs["Wk"], inputs["Wv"], inputs["Wo"]
    W1, b1, W2, b2 = inputs["W1"], f32(inputs["b1"]), inputs["W2"], f32(inputs["b2"])
    wpa, bpa = bf(inputs["Wpa"]), f32(inputs["bpa"])

    wo8 = fp8(Wo, SW)
    w1b = bf(W1)
    w2b = bf(W2)
    b1s = np.ascontiguousarray(b1)
    b2r = np.ascontiguousarray(np.asarray(b2, np.float32).reshape(1, NL * D).astype(BF16NP))

    in_maps = []
    for c in range(8):
        b, hh = c // 2, c % 2
        hsl = slice(hh * HD * KD, (hh + 1) * HD * KD)
        in_maps.append({
            "rT": np.ascontiguousarray(np.concatenate([R[b].T, ones], 0).astype(BF16NP)),
            "sT": np.ascontiguousarray(np.concatenate([s[b].T, ones], 0).astype(BF16NP)),
            "aT": np.ascontiguousarray(np.concatenate([a[b].T, ones], 0).astype(BF16NP)),
            "tix": np.ascontiguousarray(t[b].astype(np.int32).reshape(L, 1)),
            "emb": emb,
            "wr": bf(wr), "ws": bf(ws), "wa": bf(wa),
            "lng": lng, "lnb": lnb,
            "wq": np.ascontiguousarray(np.asarray(Wq, np.float32)[:, :, hsl].astype(BF16NP)),
            "wk": np.ascontiguousarray(np.asarray(Wk, np.float32)[:, :, hsl].astype(BF16NP)),
            "wv": np.ascontiguousarray(np.asarray(Wv, np.float32)[:, :, hsl].astype(BF16NP)),
            "wo": wo8,
            "w1": w1b, "b1s": b1s,
            "w2": w2b, "b2r": b2r,
            "wpa": wpa,
            "bpa": bpa.reshape(1, ACT_DIM),
        })
    return in_maps


def run_on_device(inputs, trace=False):
    trivial_gb = all(
        bool(np.all(np.asarray(inputs[k], np.float32) == 1.0)) for k in ("ln0_g", "ln1_g", "ln2_g")
    ) and all(
        bool(np.all(np.asarray(inputs[k], np.float32) == 0.0)) for k in ("ln0_b", "ln1_b", "ln2_b")
    )
    nc = _get_nc(trivial_gb)
    in_maps = _make_in_maps(inputs)
    res = run_bass_kernel_spmd(nc, in_maps, core_ids=list(range(8)), trace=trace)
    out = np.stack([res.results[2 * b]["outT"].T for b in range(N)], 0)
    return out.astype(np.float32), res


def kernel(**inputs):
    try:
        out, _ = run_on_device(inputs, trace=False)
    except Exception:
        # transient device errors usually clear on retry
        out, _ = run_on_device(inputs, trace=False)
    return out


# revision 40
# speedup vs baseline: 1.0233x; 1.0233x over previous
"""Decision Transformer on 8 Trainium2 NeuronCores.

Sharding: batch(4) x 2-way tensor parallel (attention-head split; FFN and Wo
computed redundantly on both pair cores to avoid AllReduce). Core c: batch
c//2, head-shard c%2. All cores run one SPMD instruction stream; shard
differences live in the input data. One fp8 pair-AllGather per (layer, chunk).

On-chip layout: residual stream x is transposed ([D, S]) in bf16; a quantized
fp8 copy xh feeds all weight matmuls, which run in fp8 DoubleRow mode
(2 k-tiles per matmul, 0.5 PE cycles/row). Weights are host-prescaled into
fp8 range and rescaled on PSUM evacuation. LayerNorm stats use ones-vector
matmuls; normalization applies rank-1 broadcast tiles (rstd, mean*rstd) plus
a per-partition gain/bias pass. Attention keeps logits [key, query], windows
the exp/causal-mask work to the live region, and defers softmax normalization
via a ones-column in V. Residual adds are fused into PSUM evacuation with
scalar_tensor_tensor. Emission is software-pipelined across layers so
collectives hide under the next chunk's compute.
"""

import numpy as np

import concourse.bass as bass
import concourse.mybir as mybir
import concourse.tile as tile
from concourse import bacc
from concourse.bass_utils import run_bass_kernel_spmd
from concourse.masks import make_identity

F32 = mybir.dt.float32
BF16 = mybir.dt.bfloat16
FP8 = mybir.dt.float8e4
I32 = mybir.dt.int32
AF = mybir.ActivationFunctionType
OP = mybir.AluOpType
DR = mybir.MatmulPerfMode.DoubleRow

N, L, D = 4, 512, 768
STATE, ACT_DIM = 17, 6
H, KD = 12, 64
FF = 2048
NL = 4
MAXT = 4096

S = 3 * L            # 1536 tokens
DT = D // 128        # 6 d-tiles
DP = DT // 2         # 3 d-tile pairs
CW = 512             # chunk width (tokens)
NCH = S // CW        # 3 chunks
KT = S // 128        # 12 k-tiles
HD = H // 2          # 6 heads per core
HP = HD // 2         # 3 head pairs
FFC = FF // 128      # 16 ff col-tiles
EPS = 1e-5

SW = 64.0            # prescale for wq/wk/wv/wo/w2
SW1 = 32.0           # prescale for w1 (relu path)

REPLICA_GROUPS = [[0, 1], [2, 3], [4, 5], [6, 7]]


def build_nc(trivial_gb=True):
    nc = bacc.Bacc("TRN2", target_bir_lowering=False, debug=False, num_devices=8)

    d_rT = nc.dram_tensor("rT", [2, L], BF16, kind="ExternalInput")
    d_sT = nc.dram_tensor("sT", [STATE + 1, L], BF16, kind="ExternalInput")
    d_aT = nc.dram_tensor("aT", [ACT_DIM + 1, L], BF16, kind="ExternalInput")
    d_tix = nc.dram_tensor("tix", [L, 1], I32, kind="ExternalInput")
    d_emb = nc.dram_tensor("emb", [MAXT, D], F32, kind="ExternalInput")
    d_wr = nc.dram_tensor("wr", [2, D], BF16, kind="ExternalInput")
    d_ws = nc.dram_tensor("ws", [STATE + 1, D], BF16, kind="ExternalInput")
    d_wa = nc.dram_tensor("wa", [ACT_DIM + 1, D], BF16, kind="ExternalInput")
    d_lng = nc.dram_tensor("lng", [9, D], F32, kind="ExternalInput")
    d_lnb = nc.dram_tensor("lnb", [9, D], F32, kind="ExternalInput")
    d_wq = nc.dram_tensor("wq", [NL, D, HD * KD], BF16, kind="ExternalInput")
    d_wk = nc.dram_tensor("wk", [NL, D, HD * KD], BF16, kind="ExternalInput")
    d_wv = nc.dram_tensor("wv", [NL, D, HD * KD], BF16, kind="ExternalInput")
    d_wo = nc.dram_tensor("wo", [NL, H * KD, D], FP8, kind="ExternalInput")
    d_w1 = nc.dram_tensor("w1", [NL, D, FF], BF16, kind="ExternalInput")
    d_b1s = nc.dram_tensor("b1s", [NL, FF], F32, kind="ExternalInput")
    d_w2 = nc.dram_tensor("w2", [NL, FF, D], BF16, kind="ExternalInput")
    d_b2r = nc.dram_tensor("b2r", [1, NL * D], BF16, kind="ExternalInput")
    d_wpa = nc.dram_tensor("wpa", [D, ACT_DIM], BF16, kind="ExternalInput")
    d_bpa = nc.dram_tensor("bpa", [1, ACT_DIM], F32, kind="ExternalInput")
    d_out = nc.dram_tensor("outT", [ACT_DIM, L], F32, kind="ExternalOutput")

    from contextlib import ExitStack
    with tile.TileContext(nc) as tc:
        with ExitStack() as _es:
            pp = _es.enter_context(tc.tile_pool(name="persist", bufs=1))
            wqkvp = _es.enter_context(tc.tile_pool(name="wqkv", bufs=1))
            wop = _es.enter_context(tc.tile_pool(name="wop", bufs=2))
            wffp = _es.enter_context(tc.tile_pool(name="wff", bufs=1))
            qtp = _es.enter_context(tc.tile_pool(name="qt", bufs=2))
            attp = _es.enter_context(tc.tile_pool(name="att", bufs=2))
            gatp = _es.enter_context(tc.tile_pool(name="gatp", bufs=2))
            posp = _es.enter_context(tc.tile_pool(name="posp", bufs=2))
            prp = _es.enter_context(tc.tile_pool(name="pr", bufs=4))
            htp = _es.enter_context(tc.tile_pool(name="htp", bufs=1))
            scr = _es.enter_context(tc.tile_pool(name="scr", bufs=3))
            abp = _es.enter_context(tc.tile_pool(name="abp", bufs=2))
            rowsp = _es.enter_context(tc.tile_pool(name="rows", bufs=3))
            smallp = _es.enter_context(tc.tile_pool(name="small", bufs=4))
            psA = _es.enter_context(tc.tile_pool(name="psA", bufs=2, space="PSUM"))
            psLG = _es.enter_context(tc.tile_pool(name="pslg", bufs=2, space="PSUM"))
            psST = _es.enter_context(tc.tile_pool(name="psst", bufs=1, space="PSUM"))
            psAB = _es.enter_context(tc.tile_pool(name="psab", bufs=1, space="PSUM"))
            psPV = _es.enter_context(tc.tile_pool(name="pspv", bufs=1, space="PSUM"))
            drp = _es.enter_context(tc.tile_pool(name="dram", bufs=3, space="DRAM"))
            # ---- persistent tiles ----
            x = pp.tile([128, DT, S], BF16)          # residual stream, transposed
            kT = pp.tile([128, HP, S], BF16)         # K^T (own heads)
            v = pp.tile([128, KT, HD * 128], FP8)    # V rows + ones col + pad per head
            xs = pp.tile([128, DT, L], BF16)         # last layer: packed state tokens
            ident = pp.tile([128, 128], F32)
            ones_col = pp.tile([128, 1], BF16)       # stat matmul lhsT
            onesP = pp.tile([1, 128], BF16)          # broadcast lhsT
            ones_row = pp.tile([1, CW], BF16)        # bias-row matmul rhs
            lng_sb = pp.tile([128, 9, DT], F32)
            lnb_sb = pp.tile([128, 9, DT], F32)
            b1s_sb = pp.tile([128, NL, FFC], F32)
            bpa_sb = pp.tile([ACT_DIM, 1], F32)
            wpa_sb = pp.tile([128, DT, ACT_DIM], BF16)
            wr_sb = pp.tile([2, D], BF16)
            ws_sb = pp.tile([STATE + 1, D], BF16)
            wa_sb = pp.tile([ACT_DIM + 1, D], BF16)
            rT_sb = pp.tile([2, L], BF16)
            sT_sb = pp.tile([STATE + 1, L], BF16)
            aT_sb = pp.tile([ACT_DIM + 1, L], BF16)

            magic_row = pp.tile([1, CW], I32)
            nc.vector.memset(magic_row, 0x5F3759DF)
            expb_sb = pp.tile([128, 1], F32)
            nc.vector.memset(expb_sb, -2.0)
            make_identity(nc, ident)
            nc.vector.memset(ones_col, 1.0)
            nc.vector.memset(onesP, 1.0)
            nc.vector.memset(ones_row, 1.0)

            pos_tiles = []
            for r in range(L // 128):
                tix_sb = smallp.tile([128, 1], I32, tag="tix", name=f"tix{r}")
                nc.sync.dma_start(out=tix_sb, in_=d_tix.ap()[r * 128 : (r + 1) * 128, :])
                pos = posp.tile([128, D], F32, tag="pos", bufs=4, name=f"pos{r}")
                nc.gpsimd.indirect_dma_start(
                    out=pos, out_offset=None, in_=d_emb.ap(),
                    in_offset=bass.IndirectOffsetOnAxis(ap=tix_sb[:, :1], axis=0))
                pos_tiles.append(pos)
            magic_row = pp.tile([1, CW], I32)
            nc.vector.memset(magic_row, 0x5F3759DF)
            make_identity(nc, ident)
            nc.vector.memset(ones_col, 1.0)
            nc.vector.memset(onesP, 1.0)
            nc.vector.memset(ones_row, 1.0)
            nc.sync.dma_start(out=wr_sb, in_=d_wr.ap())
            nc.sync.dma_start(out=ws_sb, in_=d_ws.ap())
            nc.sync.dma_start(out=wa_sb, in_=d_wa.ap())
            nc.sync.dma_start(out=rT_sb, in_=d_rT.ap())
            nc.sync.dma_start(out=sT_sb, in_=d_sT.ap())
            nc.sync.dma_start(out=aT_sb, in_=d_aT.ap())
            nc.sync.dma_start(out=lng_sb, in_=d_lng.ap().rearrange("g (t p) -> p g t", p=128))
            nc.sync.dma_start(out=lnb_sb, in_=d_lnb.ap().rearrange("g (t p) -> p g t", p=128))
            nc.sync.dma_start(out=b1s_sb, in_=d_b1s.ap().rearrange("l (t p) -> p l t", p=128))
            nc.sync.dma_start(out=bpa_sb, in_=d_bpa.ap().rearrange("o c -> c o"))
            nc.sync.dma_start(out=wpa_sb, in_=d_wpa.ap().rearrange("(t p) c -> p t c", p=128))

            nc.gpsimd.memset(v, 0.0)
            nc.gpsimd.memset(
                v.rearrange("p k (h w) -> p k h w", w=128)[:, :, :, 64:65], 1.0)

            # ---- per-layer weight tiles ----
            def load_qkv(li):
                wq_sb = wqkvp.tile([128, DT, HD * KD], BF16, tag="wq", name=f"wq{li}")
                wk_sb = wqkvp.tile([128, DT, HD * KD], BF16, tag="wk", name=f"wk{li}")
                wv_sb = wqkvp.tile([128, DT, HD * KD], BF16, tag="wv", name=f"wv{li}")
                nc.sync.dma_start(out=wq_sb, in_=d_wq.ap()[li].rearrange("(t p) c -> p t c", p=128))
                nc.sync.dma_start(out=wk_sb, in_=d_wk.ap()[li].rearrange("(t p) c -> p t c", p=128))
                nc.sync.dma_start(out=wv_sb, in_=d_wv.ap()[li].rearrange("(t p) c -> p t c", p=128))
                return wq_sb, wk_sb, wv_sb

            def load_wo(li):
                wo_sb = wop.tile([128, 2 * HP, D], FP8, tag="wo", name=f"wo{li}")
                nc.sync.dma_start(out=wo_sb, in_=d_wo.ap()[li].rearrange("(t p) c -> p t c", p=128))
                return wo_sb

            def load_ffn(li):
                w1_sb = wffp.tile([128, DT, FF], BF16, tag="w1", name=f"w1{li}")
                w2_sb = wffp.tile([128, FFC, D], BF16, tag="w2", name=f"w2{li}")
                b2l_sb = wffp.tile([1, D], BF16, tag="b2l", bufs=2, name=f"b2l{li}")
                nc.sync.dma_start(out=w1_sb, in_=d_w1.ap()[li].rearrange("(t p) c -> p t c", p=128))
                nc.sync.dma_start(out=w2_sb, in_=d_w2.ap()[li].rearrange("(t p) c -> p t c", p=128))
                nc.sync.dma_start(out=b2l_sb, in_=d_b2r.ap()[:, li * D : (li + 1) * D])
                return w1_sb, w2_sb, b2l_sb

            # ---- helpers ----
            def xcols(c):
                cs = slice(c * CW, (c + 1) * CW)
                return lambda dt: x[:, dt, cs]

            def x_state(dt):
                return x[:, dt, :].rearrange("p (j k) -> p k j", k=3)[:, 1, :]

            def layer_norm_gen(gi, xc, W):
                """xc(dt)->[128,W] bf16, normalized in place."""
                st = psST.tile([33, W], F32, tag="st", name="st")
                for dt in range(DT):
                    sq = scr.tile([128, W], BF16, tag="scr", name="sq")
                    nc.vector.tensor_tensor(out=sq, in0=xc(dt), in1=xc(dt), op=OP.mult)
                    nc.tensor.matmul(st[0:1, :], lhsT=ones_col, rhs=xc(dt),
                                     start=(dt == 0), stop=(dt == DT - 1))
                    nc.tensor.matmul(st[32:33, :], lhsT=ones_col, rhs=sq,
                                     start=(dt == 0), stop=(dt == DT - 1))
                mrow = rowsp.tile([1, W], F32, tag="rowf", name="mrow")
                nc.vector.tensor_scalar(out=mrow, in0=st[0:1, :], scalar1=1.0 / D,
                                        scalar2=None, op0=OP.mult)
                m2 = rowsp.tile([1, W], F32, tag="rowf", name="m2")
                nc.vector.tensor_tensor(out=m2, in0=mrow, in1=mrow, op=OP.mult)
                ve = rowsp.tile([1, W], F32, tag="rowf", name="ve")
                nc.vector.scalar_tensor_tensor(out=ve, in0=st[32:33, :], scalar=1.0 / D,
                                               in1=m2, op0=OP.mult, op1=OP.subtract)
                # rsqrt via bit-trick seed + 2 Newton iterations (DVE only,
                # avoids Act sqrt/ln which would thrash the activation tables)
                yi = rowsp.tile([1, W], I32, tag="rowf", name="yi")
                nc.vector.tensor_scalar(out=yi, in0=ve.bitcast(I32), scalar1=1,
                                        scalar2=None, op0=OP.logical_shift_right)
                nc.vector.tensor_tensor(out=yi, in0=magic_row[0:1, 0:W], in1=yi,
                                        op=OP.subtract)
                y = yi.bitcast(F32)
                t = rowsp.tile([1, W], F32, tag="rowf", name="t")
                nc.vector.tensor_tensor(out=t, in0=y, in1=y, op=OP.mult)
                nc.vector.tensor_tensor(out=t, in0=t, in1=ve, op=OP.mult)
                nc.vector.tensor_scalar(out=t, in0=t, scalar1=-0.5, scalar2=1.5,
                                        op0=OP.mult, op1=OP.add)
                rstd = rowsp.tile([1, W], BF16, tag="rowb", name="rstd")
                with nc.allow_low_precision(reason="bf16 rstd feeds bf16 matmul"):
                    nc.vector.tensor_tensor(out=rstd, in0=y, in1=t, op=OP.mult)
                beta = rowsp.tile([1, W], BF16, tag="rowb", name="beta")
                nc.vector.scalar_tensor_tensor(out=beta, in0=st[0:1, :], scalar=1.0 / D,
                                               in1=rstd, op0=OP.mult, op1=OP.mult)
                yield
                ab = psAB.tile([128, 2, CW], F32, tag="ab", name="ab")
                nc.tensor.matmul(ab[:, 0, 0:W], lhsT=onesP, rhs=rstd, start=True, stop=True)
                nc.tensor.matmul(ab[:, 1, 0:W], lhsT=onesP, rhs=beta, start=True, stop=True)
                a_sb = abp.tile([128, W], BF16, tag="a_sb", name="a_sb")
                b_sb = abp.tile([128, W], BF16, tag="b_sb", name="b_sb")
                nc.scalar.activation(out=a_sb, in_=ab[:, 0, 0:W], func=AF.Identity,
                                     bias=0.0, scale=1.0)
                nc.scalar.activation(out=b_sb, in_=ab[:, 1, 0:W], func=AF.Identity,
                                     bias=0.0, scale=1.0)
                yield
                for dt in range(DT):
                    if trivial_gb:
                        t1 = scr.tile([128, W], BF16, tag="scr", name="t1")
                        nc.vector.tensor_tensor(out=t1, in0=xc(dt), in1=a_sb, op=OP.mult)
                        nc.vector.tensor_tensor(out=xc(dt), in0=t1, in1=b_sb, op=OP.subtract)
                    else:
                        t1 = scr.tile([128, W], BF16, tag="scr", name="t1")
                        nc.vector.tensor_tensor(out=t1, in0=xc(dt), in1=a_sb, op=OP.mult)
                        nc.vector.tensor_tensor(out=t1, in0=t1, in1=b_sb, op=OP.subtract)
                        nc.vector.tensor_scalar(out=xc(dt), in0=t1,
                                                scalar1=lng_sb[:, gi, dt : dt + 1],
                                                scalar2=lnb_sb[:, gi, dt : dt + 1],
                                                op0=OP.mult, op1=OP.add)
                    if dt % 3 == 2:
                        yield

            def kv_gen(wk_sb, wv_sb, c):
                cs = slice(c * CW, (c + 1) * CW)
                for hp in range(HP):
                    pk = psA.tile([128, CW], F32, tag="mm", name="pk")
                    for dt in range(DT):
                        nc.tensor.matmul(pk, lhsT=wk_sb[:, dt, hp * 128 : (hp + 1) * 128],
                                         rhs=x[:, dt, cs],
                                         start=(dt == 0), stop=(dt == DT - 1))
                    nc.scalar.activation(out=kT[:, hp, cs], in_=pk, func=AF.Identity,
                                         bias=0.0, scale=1.0)
                    yield
                for j in range(4):
                    kt = 4 * c + j
                    pv_ = psA.tile([128, HD * KD], F32, tag="mm", name="pv_")
                    for dt in range(DT):
                        nc.tensor.matmul(pv_, lhsT=x[:, dt, kt * 128 : (kt + 1) * 128],
                                         rhs=wv_sb[:, dt, :],
                                         start=(dt == 0), stop=(dt == DT - 1))
                    nc.scalar.activation(
                        out=v[:, kt, :].rearrange("p (h w) -> p h w", w=128)[:, :, 0:64],
                        in_=pv_.rearrange("p (h w) -> p h w", w=64),
                        func=AF.Identity, bias=0.0, scale=1.0)
                    if j % 2 == 1:
                        yield

            def attn_gen(wq_sb, qsrc, npair, masks, W, out_attnT):
                """masks: list per kt of None or (dead, w0, base, step) for the
                affine window [w0, w0+winw) with iota = base + step*i - p."""
                qTc = qtp.tile([128, HP, W], BF16, tag="qT", name="qTc")
                for hp in range(HP):
                    pq = psA.tile([128, W], F32, tag="mm", name="pq")
                    for dt in range(DT):
                        nc.tensor.matmul(pq, lhsT=wq_sb[:, dt, hp * 128 : (hp + 1) * 128],
                                         rhs=qsrc(dt), start=(dt == 0), stop=(dt == DT - 1))
                    nc.vector.tensor_copy(out=qTc[:, hp, :], in_=pq)
                yield
                for hd in range(HD):
                    hp, hi = hd // 2, hd % 2
                    prow = slice(64 * hi, 64 * hi + 64)
                    pv = psPV.tile([128, W], F32, tag="pv", name="pv")
                    for ip in range(npair):
                        pr2 = prp.tile([128, 2, W], FP8, tag="pr", name="pr2")
                        for j in (0, 1):
                            kt = 2 * ip + j
                            lgp = psLG.tile([128, CW], F32, tag="lg", name="lgp")
                            nc.tensor.matmul(lgp[:, 0:W],
                                             lhsT=kT[prow, hp, kt * 128 : (kt + 1) * 128],
                                             rhs=qTc[prow, hp, :], start=True, stop=True)
                            m_ = masks[kt]
                            dead, w0, bse, stp, winw = (0, 0, 0, 1, 0) if m_ is None else m_
                            if dead > 0:
                                nc.gpsimd.memset(pr2[:, j, 0:dead], 0.0)
                            if dead < W:
                                nc.scalar.activation(out=pr2[:, j, dead:W],
                                                     in_=lgp[:, dead:W], func=AF.Exp,
                                                     bias=expb_sb, scale=float(KD) ** -0.5)
                            if winw > 0:
                                nc.gpsimd.affine_select(
                                    out=pr2[:, j, w0 : w0 + winw],
                                    in_=pr2[:, j, w0 : w0 + winw],
                                    compare_op=OP.is_ge, fill=0.0,
                                    base=bse, channel_multiplier=-1,
                                    pattern=[[stp, winw]])
                        nc.tensor.matmul(pv, lhsT=v[:, 2 * ip : 2 * ip + 2, hd * 128 : (hd + 1) * 128],
                                         rhs=pr2, start=(ip == 0), stop=(ip == npair - 1),
                                         perf_mode=DR)
                    nc.scalar.activation(out=out_attnT[prow, hp, :], in_=pv[0:64, :],
                                         func=AF.Identity, bias=0.0, scale=1.0)
                    rc = rowsp.tile([1, W], BF16, tag="rowb", name="rc")
                    with nc.allow_low_precision(reason="bf16 softmax denom recip"):
                        nc.vector.reciprocal(out=rc, in_=pv[64:65, :])
                    bc = psA.tile([64, W], F32, tag="mm", name="bc")
                    nc.tensor.matmul(bc, lhsT=onesP[:, 0:64], rhs=rc, start=True, stop=True)
                    nc.vector.tensor_tensor(out=out_attnT[prow, hp, :],
                                            in0=out_attnT[prow, hp, :],
                                            in1=bc, op=OP.mult)
                    yield

            def stage_allgather(attnT, W):
                ag_in = drp.tile([HP * 128, W], FP8, tag="agin", name="ag_in")
                ag_o = drp.tile([2 * HP * 128, W], FP8, tag="agout", name="ag_o")
                nc.sync.dma_start(out=ag_in.rearrange("(t p) c -> p t c", p=128), in_=attnT)
                nc.gpsimd.collective_compute(
                    "AllGather", OP.bypass, replica_groups=REPLICA_GROUPS,
                    ins=[ag_in.opt()], outs=[ag_o.opt()])
                return ag_o

            # full-layer causal masks per chunk: diagonal k-tiles get a
            # 128-wide affine window at w0=(kt-4c)*128, iota = i - p
            def chunk_masks(c):
                ms = []
                for kt in range(4 * (c + 1)):
                    if kt >= 4 * c:
                        w0 = (kt - 4 * c) * 128
                        ms.append((w0, w0, 0, 1, min(128, CW - w0)))
                    else:
                        ms.append(None)
                return ms

            def wo_residual_gen(wo_sb, ag_o, xc, W):
                gat = gatp.tile([128, 2 * HP, W], FP8, tag="gat", name="gat")
                nc.sync.dma_start(out=gat, in_=ag_o.rearrange("(t p) c -> p t c", p=128))
                for dc in range(DT):
                    py = psA.tile([128, W], F32, tag="mm", name="py")
                    for h2 in range(HP):
                        nc.tensor.matmul(py, lhsT=wo_sb[:, 2 * h2 : 2 * h2 + 2, dc * 128 : (dc + 1) * 128],
                                         rhs=gat[:, 2 * h2 : 2 * h2 + 2, :],
                                         start=(h2 == 0), stop=(h2 == HP - 1), perf_mode=DR)
                    nc.vector.scalar_tensor_tensor(out=xc(dc), in0=py, scalar=1.0 / SW,
                                                   in1=xc(dc), op0=OP.mult, op1=OP.add)
                    if dc % 2 == 1:
                        yield

            def ffn_gen(li, w1_sb, w2_sb, b2l_sb, xc, W):
                ht = htp.tile([128, FFC, W], BF16, tag="ht", name="ht")
                for f in range(FFC):
                    ph = psA.tile([128, W], F32, tag="mm", name="ph")
                    for dt in range(DT):
                        nc.tensor.matmul(ph, lhsT=w1_sb[:, dt, f * 128 : (f + 1) * 128],
                                         rhs=xc(dt), start=(dt == 0), stop=(dt == DT - 1))
                    nc.scalar.activation(out=ht[:, f, :], in_=ph, func=AF.Relu,
                                         bias=b1s_sb[:, li, f : f + 1], scale=1.0)
                    if f % 4 == 3:
                        yield
                for dc in range(DT):
                    ps2 = psA.tile([128, W], F32, tag="mm", name="ps2")
                    nc.tensor.matmul(ps2, lhsT=b2l_sb[0:1, dc * 128 : (dc + 1) * 128],
                                     rhs=ones_row[0:1, 0:W], start=True, stop=False)
                    for f in range(FFC):
                        nc.tensor.matmul(ps2, lhsT=w2_sb[:, f, dc * 128 : (dc + 1) * 128],
                                         rhs=ht[:, f, :],
                                         start=False, stop=(f == FFC - 1))
                    nc.vector.scalar_tensor_tensor(out=xc(dc), in0=ps2, scalar=1.0,
                                                   in1=xc(dc), op0=OP.mult, op1=OP.add)
                    if dc % 2 == 1:
                        yield

            def mlp_gen(li, wo_sb, wff3, ag_o, xc, W):
                yield from wo_residual_gen(wo_sb, ag_o, xc, W)
                yield from layer_norm_gen(1 + li, xc, W)
                yield from ffn_gen(li, wff3[0], wff3[1], wff3[2], xc, W)
                yield from layer_norm_gen(5 + li, xc, W)

            def drain(g):
                for _ in g:
                    pass

            def co_emit(*specs):
                # specs: (generator, start_round) pairs, round-robin emission
                gens = [[g, start, False] for g, start in specs]
                rnd = 0
                while not all(g[2] for g in gens):
                    for g in gens:
                        if g[2] or rnd < g[1]:
                            continue
                        try:
                            next(g[0])
                        except StopIteration:
                            g[2] = True
                    rnd += 1

            # ---- embedding ----
            wq0, wk0, wv0 = load_qkv(0)
            wo0 = load_wo(0)
            wff0 = load_ffn(0)

            def x_kind(dt, kind):
                return x[:, dt, :].rearrange("p (j k) -> p k j", k=3)[:, kind, :]

            for dt in range(DT):
                for w_sb, t_sb, kind in ((wr_sb, rT_sb, 0), (ws_sb, sT_sb, 1), (wa_sb, aT_sb, 2)):
                    pe = psA.tile([128, L], F32, tag="mm", name="pe")
                    for r in range(L // 128):
                        nc.tensor.matmul(pe[:, r * 128 : (r + 1) * 128],
                                         lhsT=pos_tiles[r][:, dt * 128 : (dt + 1) * 128],
                                         rhs=ident, start=(r == 0), stop=False,
                                         is_transpose=True)
                    nc.tensor.matmul(pe, lhsT=w_sb[:, dt * 128 : (dt + 1) * 128], rhs=t_sb,
                                     start=False, stop=True)
                    nc.scalar.activation(out=x_kind(dt, kind), in_=pe, func=AF.Identity,
                                         bias=0.0, scale=1.0)

            # ---- pipelined layers ----
            # Two concurrent emission streams per phase-pair (attention of one
            # chunk interleaved with the MLP of another) so every engine's
            # 4-deep wait-queue window always holds ready work.
            QKV = {0: (wq0, wk0, wv0)}
            WO = {0: wo0}
            W12 = {0: wff0}
            ag_pending = {}
            ag_s = {}
            HWL = CW // 2

            def chain2(*gens):
                for g in gens:
                    yield from g

            def attn_stage_gen(li, c):
                attnT = attp.tile([128, HP, CW], FP8, tag="attnT", name="attnT")
                yield from attn_gen(QKV[li][0], xcols(c), 2 * (c + 1), chunk_masks(c), CW, attnT)
                ag_pending[(li, c)] = stage_allgather(attnT, CW)

            def state_masks(hw0, W, nkt):
                ms = []
                for kt in range(nkt):
                    t0 = max(0, (kt * 128 + 1) // 3)     # first possibly-live col (global)
                    dead = min(W, max(0, t0 - hw0))
                    w0 = dead
                    winw = min(44, W - w0)
                    base = 3 * (hw0 + w0) - kt * 128 + 1
                    if dead >= W:
                        ms.append((W, 0, 0, 3, 0))
                    else:
                        ms.append((dead, w0, base, 3, winw))
                return ms

            def compact_gen(hw0, W):
                hs = slice(hw0, hw0 + W)
                for dt in range(DT):
                    nc.gpsimd.tensor_copy(out=xs[:, dt, hs], in_=x_state(dt)[:, hs])
                    if dt % 3 == 2:
                        yield

            def attn_state_gen(half):
                hw0 = half * HWL
                hs = slice(hw0, hw0 + HWL)
                nkt = 6 if half == 0 else 12
                li = NL - 1
                attnT = attp.tile([128, HP, HWL], FP8, tag="attnTs", name="attnTs")
                yield from attn_gen(QKV[li][0], (lambda dt: xs[:, dt, hs]),
                                    nkt // 2, state_masks(hw0, HWL, nkt), HWL, attnT)
                ag_s[half] = stage_allgather(attnT, HWL)

            def mlp_state_head_gen(half):
                li = NL - 1
                hw0 = half * HWL
                hs = slice(hw0, hw0 + HWL)
                xc = lambda dt: xs[:, dt, hs]
                yield from mlp_gen(li, WO[li], W12[li], ag_s[half], xc, HWL)
                po = psA.tile([ACT_DIM, HWL], F32, tag="mm", name="po")
                for dt in range(DT):
                    nc.tensor.matmul(po, lhsT=wpa_sb[:, dt, :], rhs=xs[:, dt, hs],
                                     start=(dt == 0), stop=(dt == DT - 1))
                ot = scr.tile([ACT_DIM, HWL], F32, tag="ot", bufs=2, name="ot")
                nc.scalar.activation(out=ot, in_=po, func=AF.Identity, bias=bpa_sb, scale=1.0)
                nc.sync.dma_start(out=d_out.ap()[:, hs], in_=ot)

            drain(layer_norm_gen(0, xcols(0), CW))
            drain(kv_gen(wk0, wv0, 0))
            co_emit((attn_stage_gen(0, 0), 0),
                    (chain2(layer_norm_gen(0, xcols(1), CW),
                            kv_gen(wk0, wv0, 1),
                            layer_norm_gen(0, xcols(2), CW)), 0))

            for li in range(NL - 1):
                co_emit((attn_stage_gen(li, 1), 0),
                        (mlp_gen(li, WO[li], W12[li], ag_pending[(li, 0)], xcols(0), CW), 3),
                        (kv_gen(QKV[li][1], QKV[li][2], 2), 0))
                QKV[li + 1] = load_qkv(li + 1)
                WO[li + 1] = load_wo(li + 1)
                co_emit((attn_stage_gen(li, 2), 0),
                        (mlp_gen(li, WO[li], W12[li], ag_pending[(li, 1)], xcols(1), CW), 3))
                W12[li + 1] = load_ffn(li + 1)
                if li + 1 < NL - 1:
                    co_emit((chain2(kv_gen(QKV[li + 1][1], QKV[li + 1][2], 0),
                                    attn_stage_gen(li + 1, 0)), 0),
                            (mlp_gen(li, WO[li], W12[li], ag_pending[(li, 2)], xcols(2), CW), 3),
                            (kv_gen(QKV[li + 1][1], QKV[li + 1][2], 1), 0))
                else:
                    lz = li + 1
                    co_emit((chain2(kv_gen(QKV[lz][1], QKV[lz][2], 0),
                                    kv_gen(QKV[lz][1], QKV[lz][2], 1),
                                    compact_gen(0, HWL),
                                    attn_state_gen(0)), 0),
                            (mlp_gen(li, WO[li], W12[li], ag_pending[(li, 2)], xcols(2), CW), 3))
                    co_emit((chain2(kv_gen(QKV[lz][1], QKV[lz][2], 2),
                                    compact_gen(HWL, HWL),
                                    attn_state_gen(1)), 0),
                            (mlp_state_head_gen(0), 3))
                    drain(mlp_state_head_gen(1))

    nc.compile()
    return nc


_NC_CACHE = {}


def _get_nc(trivial_gb=True):
    if trivial_gb not in _NC_CACHE:
        _NC_CACHE[trivial_gb] = build_nc(trivial_gb)
    return _NC_CACHE[trivial_gb]


def _make_in_maps(inputs):
    import ml_dtypes
    FP8NP = ml_dtypes.float8_e4m3
    BF16NP = ml_dtypes.bfloat16

    f32 = lambda a: np.ascontiguousarray(np.asarray(a, dtype=np.float32))
    bf = lambda a: np.ascontiguousarray(np.asarray(a, dtype=np.float32).astype(BF16NP))
    fp8 = lambda a, s: np.ascontiguousarray(
        (np.asarray(a, dtype=np.float32) * s).astype(FP8NP))

    R, s, a, t = f32(inputs["R"]), f32(inputs["s"]), f32(inputs["a"]), np.asarray(inputs["t"])
    ones = np.ones((1, L), np.float32)
    lng = np.concatenate([f32(inputs["ln0_g"])[None], f32(inputs["ln1_g"]), f32(inputs["ln2_g"])], 0)
    lnb = np.concatenate([f32(inputs["ln0_b"])[None], f32(inputs["ln1_b"]), f32(inputs["ln2_b"])], 0)
    wr = np.concatenate([f32(inputs["Wr"]), f32(inputs["br"])[None]], 0)
    ws = np.concatenate([f32(inputs["Ws"]), f32(inputs["bs"])[None]], 0)
    wa = np.concatenate([f32(inputs["Wa"]), f32(inputs["ba"])[None]], 0)
    emb = f32(inputs["embed_t"])
    Wq, Wk, Wv, Wo = inputs["Wq"], inputs["Wk"], inputs["Wv"], inputs["Wo"]
    W1, b1, W2, b2 = inputs["W1"], f32(inputs["b1"]), inputs["W2"], f32(inputs["b2"])
    wpa, bpa = bf(inputs["Wpa"]), f32(inputs["bpa"])

    wo8 = fp8(Wo, SW)
    w1b = bf(W1)
    w2b = bf(W2)
    b1s = np.ascontiguousarray(b1)
    b2r = np.ascontiguousarray(np.asarray(b2, np.float32).reshape(1, NL * D).astype(BF16NP))

    in_maps = []
    for c in range(8):
        b, hh = c // 2, c % 2
        hsl = slice(hh * HD * KD, (hh + 1) * HD * KD)
        in_maps.append({
            "rT": np.ascontiguousarray(np.concatenate([R[b].T, ones], 0).astype(BF16NP)),
            "sT": np.ascontiguousarray(np.concatenate([s[b].T, ones], 0).astype(BF16NP)),
            "aT": np.ascontiguousarray(np.concatenate([a[b].T, ones], 0).astype(BF16NP)),
            "tix": np.ascontiguousarray(t[b].astype(np.int32).reshape(L, 1)),
            "emb": emb,
            "wr": bf(wr), "ws": bf(ws), "wa": bf(wa),
            "lng": lng, "lnb": lnb,
            "wq": np.ascontiguousarray(np.asarray(Wq, np.float32)[:, :, hsl].astype(BF16NP)),
            "wk": np.ascontiguousarray(np.asarray(Wk, np.float32)[:, :, hsl].astype(BF16NP)),
            "wv": np.ascontiguousarray(np.asarray(Wv, np.float32)[:, :, hsl].astype(BF16NP)),
            "wo": wo8,
            "w1": w1b, "b1s": b1s,
            "w2": w2b, "b2r": b2r,
            "wpa": wpa,
            "bpa": bpa.reshape(1, ACT_DIM),
        })
    return in_maps


def run_on_device(inputs, trace=False):
    trivial_gb = all(
        bool(np.all(np.asarray(inputs[k], np.float32) == 1.0)) for k in ("ln0_g", "ln1_g", "ln2_g")
    ) and all(
        bool(np.all(np.asarray(inputs[k], np.float32) == 0.0)) for k in ("ln0_b", "ln1_b", "ln2_b")
    )
    nc = _get_nc(trivial_gb)
    in_maps = _make_in_maps(inputs)
    res = run_bass_kernel_spmd(nc, in_maps, core_ids=list(range(8)), trace=trace)
    out = np.stack([res.results[2 * b]["outT"].T for b in range(N)], 0)
    return out.astype(np.float32), res


def kernel(**inputs):
    try:
        out, _ = run_on_device(inputs, trace=False)
    except Exception:
        # transient device errors usually clear on retry
        out, _ = run_on_device(inputs, trace=False)
    return out
